# revision 1
# baseline (speedup 1.0000x reference)
"""Multi-head attention (B=4, S=2048, D=1024, H=16) on 8 TRN2 NeuronCores.

Sharding: core c = 2*b + g handles batch b (of 4) and head-group g (of 2,
8 heads / 512 model dims each).  Per core (all matmuls bf16, fp32 PSUM):
  - QKV projections for its batch restricted to its 512 output dims;
    qhT/khT [512, 2048] and vh [2048, 520] stay resident in SBUF
  - attention for its 8 heads in transposed-scores layout (scoresT[k, q]):
    softmax denominator via a ones-column appended to V; no max subtraction
    (scores are ~N(0, 0.08^2) after the 1/32 scale, exp cannot overflow);
    bf16 rounding of Q/K is benign because score errors enter exp()
    as tiny absolute perturbations
  - output projection partial over its 512 model dims, interleaved with the
    second half of attention; partials ReduceScatter'd pairwise in 8 chunks
    so the collective overlaps compute
Host: pre-transposes inputs/weights (bf16), feeds per-core shards, and
reassembles the full [4, 2048, 1024] fp32 output from the 8 per-core
[1024, 1024] outputs (chunked-RS row interleaving: core 2b+g holds rows
256*ch + [128*g, 128*(g+1)) of batch b for ch in 0..7).
"""

import numpy as np
import ml_dtypes

import concourse.bass as bass
import concourse.mybir as mybir
import concourse.tile as tile
from concourse import bacc
from concourse.bass_utils import run_bass_kernel_spmd

N_CORES = 8
S = 2048          # sequence length
D = 1024          # d_model
DL = 512          # local model dims (8 heads x 64)
NH = 8            # local heads
DH = 64           # head dim
SCALE = 1.0 / 32.0  # 1/sqrt(d_model)

F32 = mybir.dt.float32
F32R = mybir.dt.float32r
BF16 = mybir.dt.bfloat16

_NC_CACHE = None


def _build_nc(repeat=1, phases="abc", collective=True, overlap_c=True):
    nc = bacc.Bacc("TRN2", target_bir_lowering=False, debug=False,
                   num_devices=N_CORES)

    xq = nc.dram_tensor("xq", [D, S], BF16, kind="ExternalInput")
    xk = nc.dram_tensor("xk", [D, S], BF16, kind="ExternalInput")
    xv = nc.dram_tensor("xv", [D, S], BF16, kind="ExternalInput")
    wqt = nc.dram_tensor("wqt", [D, DL], BF16, kind="ExternalInput")
    wkt = nc.dram_tensor("wkt", [D, DL], BF16, kind="ExternalInput")
    wvt = nc.dram_tensor("wvt", [D, DL], BF16, kind="ExternalInput")
    wot = nc.dram_tensor("wot", [DL, D], BF16, kind="ExternalInput")
    y = nc.dram_tensor("y", [S // 2, D], F32, kind="ExternalOutput")

    ypart = nc.dram_tensor("ypart", [S, D], F32)
    yrs = nc.dram_tensor("yrs", [S // 2, D], F32)

    with tile.TileContext(nc) as tc:
        with (
            tc.tile_pool(name="big", bufs=20) as big,        # x chunks / attn_outT
            tc.tile_pool(name="wp", bufs=2) as wpool,       # wq/wk/wv (sequential)
            tc.tile_pool(name="wop", bufs=1) as wopool,     # woT
            tc.tile_pool(name="vhp", bufs=16) as vhp,       # vh | ones
            tc.tile_pool(name="expp", bufs=6) as expp,      # exp(scores)
            tc.tile_pool(name="pvsp", bufs=4) as pvsp,      # pv psum drain
            tc.tile_pool(name="rcp", bufs=4) as rcp,        # reciprocal row
            tc.tile_pool(name="rbp", bufs=4) as rbp,        # bcast reciprocal
            tc.tile_pool(name="stgp", bufs=4) as stgp,      # psum->dram staging
            tc.tile_pool(name="ps", bufs=4, space="PSUM") as ps,
        ):
            for rep in range(repeat):
                pfx = f"r{rep}_"
                # woT load (bf16): [512, 1024] -> [128, 4, 1024]
                wo_sb = wopool.tile([128, 4, D], BF16, tag="wo", name=f"{pfx}wo_sb")
                nc.sync.dma_start(
                    out=wo_sb[:], in_=wot[:].rearrange("(t p) n -> p t n", p=128)
                )

                # ---------------- Phase A: projections (V, K, Q) ----------
                # A-v: vh[seq_block, dl] with a ones column per head slot.
                w_sb = wpool.tile([128, 8, DL], BF16, tag="w", name=f"{pfx}w_v")
                nc.sync.dma_start(
                    out=w_sb[:], in_=wvt[:].rearrange("(kc p) m -> p kc m", p=128)
                )
                x_sb = []
                for kc in range(8):
                    xt = big.tile([128, S], BF16, tag="big", name=f"{pfx}xv_{kc}")
                    nc.sync.dma_start(out=xt[:], in_=xv[kc * 128:(kc + 1) * 128, :])
                    x_sb.append(xt)
                vh_sb = []
                for st in range(16):
                    acc = ps.tile([128, 512], F32, tag="ps", name=f"{pfx}psv_{st}")
                    for kc in range(8):
                        nc.tensor.matmul(
                            acc[:],
                            x_sb[kc][:, st * 128:(st + 1) * 128],
                            w_sb[:, kc, :],
                            start=(kc == 0),
                            stop=(kc == 7),
                        )
                    vt = vhp.tile([128, NH, DH + 1], BF16, tag="vh", name=f"{pfx}vh_{st}")
                    nc.vector.tensor_copy(
                        vt[:, :, 0:DH], acc[:].rearrange("p (h d) -> p h d", d=DH)
                    )
                    nc.vector.memset(vt[:, :, DH:DH + 1], 1.0)
                    vh_sb.append(vt)

                # A-k / A-q: out[dl_block, seq] = sum_kc wT[kc,dl].T @ xT[kc,seq]
                # results stay resident in SBUF: tile mc holds dl rows
                # [128*mc, 128*(mc+1)) = heads 2mc, 2mc+1.
                khT_sb, qhT_sb = [], []
                for name, wdram, xdram, dest in (
                    ("k", wkt, xk, khT_sb),
                    ("q", wqt, xq, qhT_sb),
                ):
                    w_sb = wpool.tile([128, 8, DL], BF16, tag="w", name=f"{pfx}w_{name}")
                    nc.sync.dma_start(
                        out=w_sb[:],
                        in_=wdram[:].rearrange("(kc p) m -> p kc m", p=128),
                    )
                    x_sb = []
                    for kc in range(8):
                        xt = big.tile([128, S], BF16, tag="big", name=f"{pfx}x{name}_{kc}")
                        nc.sync.dma_start(out=xt[:], in_=xdram[kc * 128:(kc + 1) * 128, :])
                        x_sb.append(xt)
                    for mc in range(4):
                        pt = big.tile([128, S], BF16, tag="big",
                                      name=f"{pfx}{name}hT_{mc}")
                        dest.append(pt)
                        for nt in range(4):
                            acc = ps.tile([128, 512], F32, tag="ps",
                                          name=f"{pfx}ps{name}_{mc}_{nt}")
                            for kc in range(8):
                                nc.tensor.matmul(
                                    acc[:],
                                    w_sb[:, kc, mc * 128:(mc + 1) * 128],
                                    x_sb[kc][:, nt * 512:(nt + 1) * 512],
                                    start=(kc == 0),
                                    stop=(kc == 7),
                                )
                            nc.vector.tensor_copy(
                                pt[:, nt * 512:(nt + 1) * 512], acc[:]
                            )

                # ---------------- Phase B: attention ----------------
                if "b" not in phases:
                    continue
                attn_sb = [
                    big.tile([128, S], BF16, tag="big", name=f"{pfx}attn_{t}")
                    for t in range(4)
                ]
                def emit_c_half(co):
                    # output projection + chunked ReduceScatter for q rows
                    # [1024*co, 1024*(co+1))
                    if "c" not in phases:
                        return
                    for qb in range(8 * co, 8 * (co + 1)):
                        for nt in range(2):
                            acc = ps.tile([128, 512], F32, tag="ps",
                                          name=f"{pfx}psy_{qb}_{nt}")
                            for t in range(4):
                                nc.tensor.matmul(
                                    acc[:],
                                    attn_sb[t][:, qb * 128:(qb + 1) * 128],
                                    wo_sb[:, t, nt * 512:(nt + 1) * 512],
                                    start=(t == 0),
                                    stop=(t == 3),
                                )
                            st = stgp.tile([128, 512], F32, tag="ystg",
                                           name=f"{pfx}sty_{qb}_{nt}")
                            nc.vector.tensor_copy(st[:], acc[:])
                            nc.sync.dma_start(
                                out=ypart[qb * 128:(qb + 1) * 128,
                                          nt * 512:(nt + 1) * 512],
                                in_=st[:],
                            )
                        if qb % 2 == 1:
                            ch = qb // 2
                            if collective:
                                nc.gpsimd.collective_compute(
                                    "ReduceScatter",
                                    mybir.AluOpType.add,
                                    replica_groups=[[0, 1], [2, 3], [4, 5], [6, 7]],
                                    ins=[ypart[256 * ch:256 * (ch + 1), :].opt()],
                                    outs=[yrs[128 * ch:128 * (ch + 1), :].opt()],
                                )
                                nc.sync.dma_start(
                                    out=y[128 * ch:128 * (ch + 1), :],
                                    in_=yrs[128 * ch:128 * (ch + 1), :],
                                )
                            elif ch < 4:
                                nc.sync.dma_start(
                                    out=y[256 * ch:256 * (ch + 1), :],
                                    in_=ypart[256 * ch:256 * (ch + 1), :],
                                )

                for qt in range(2):
                    for h in range(NH):
                        t, p = h // 2, h % 2
                        kh = khT_sb[t]
                        qsl = qhT_sb[t]
                        pv = ps.tile([65, 1024], F32, tag="ps", name=f"{pfx}pv_{h}_{qt}")
                        for kb in range(16):
                            sc = ps.tile([128, 1024], F32, tag="ps",
                                         name=f"{pfx}sc_{h}_{qt}_{kb}")
                            for half in range(2):
                                nc.tensor.matmul(
                                    sc[:, half * 512:(half + 1) * 512],
                                    kh[64 * p:64 * p + 64, kb * 128:(kb + 1) * 128],
                                    qsl[64 * p:64 * p + 64,
                                        qt * 1024 + half * 512:
                                        qt * 1024 + (half + 1) * 512],
                                    start=True,
                                    stop=True,
                                )
                            ex = expp.tile([128, 1024], BF16, tag="exp",
                                           name=f"{pfx}ex_{h}_{qt}_{kb}")
                            nc.scalar.activation(
                                ex[:], sc[:], mybir.ActivationFunctionType.Exp,
                                scale=SCALE,
                            )
                            for half in range(2):
                                nc.tensor.matmul(
                                    pv[:, half * 512:(half + 1) * 512],
                                    vh_sb[kb][:, h, :],
                                    ex[:, half * 512:(half + 1) * 512],
                                    start=(kb == 0),
                                    stop=(kb == 15),
                                )
                        pvs = pvsp.tile([65, 1024], F32, tag="pvs",
                                        name=f"{pfx}pvs_{h}_{qt}")
                        nc.vector.tensor_copy(pvs[:], pv[:])
                        for half in range(2):
                            hs = slice(half * 512, (half + 1) * 512)
                            rc = rcp.tile([1, 512], F32, tag="rc",
                                          name=f"{pfx}rc_{h}_{qt}_{half}")
                            nc.vector.reciprocal(rc[:], pvs[64:65, hs])
                            rb = rbp.tile([64, 512], F32, tag="rb",
                                          name=f"{pfx}rb_{h}_{qt}_{half}")
                            nc.gpsimd.partition_broadcast(rb[:], rc[:])
                            dst = slice(qt * 1024 + half * 512,
                                        qt * 1024 + (half + 1) * 512)
                            nc.vector.tensor_mul(
                                attn_sb[t][64 * p:64 * p + 64, dst],
                                pvs[0:64, hs], rb[:]
                            )
                    # phase C for this q half overlaps the next qt's attention
                    if overlap_c:
                        emit_c_half(qt)
                if not overlap_c:
                    emit_c_half(0)
                    emit_c_half(1)

    nc.finalize()
    return nc


def _get_nc():
    global _NC_CACHE
    if _NC_CACHE is None:
        _NC_CACHE = _build_nc()
    return _NC_CACHE


def kernel(q, k, v, wq, wk, wv, wo, _res_hook=None):
    q = np.asarray(q, dtype=np.float32)
    k = np.asarray(k, dtype=np.float32)
    v = np.asarray(v, dtype=np.float32)
    wq = np.asarray(wq, dtype=np.float32)
    wk = np.asarray(wk, dtype=np.float32)
    wv = np.asarray(wv, dtype=np.float32)
    wo = np.asarray(wo, dtype=np.float32)
    B = q.shape[0]

    nc = _get_nc()
    in_maps = []
    for c in range(N_CORES):
        b, g = c // 2, c % 2
        sl = slice(DL * g, DL * (g + 1))
        in_maps.append({
            "xq": np.ascontiguousarray(q[b].T).astype(ml_dtypes.bfloat16),
            "xk": np.ascontiguousarray(k[b].T).astype(ml_dtypes.bfloat16),
            "xv": np.ascontiguousarray(v[b].T).astype(ml_dtypes.bfloat16),
            "wqt": np.ascontiguousarray(wq[sl, :].T).astype(ml_dtypes.bfloat16),
            "wkt": np.ascontiguousarray(wk[sl, :].T).astype(ml_dtypes.bfloat16),
            "wvt": np.ascontiguousarray(wv[sl, :].T).astype(ml_dtypes.bfloat16),
            "wot": np.ascontiguousarray(wo[:, sl].T).astype(ml_dtypes.bfloat16),
        })

    res = run_bass_kernel_spmd(nc, in_maps, list(range(N_CORES)))
    if _res_hook is not None:
        _res_hook(res)

    out = np.empty((B, S, D), dtype=np.float32)
    for c in range(N_CORES):
        b, g = c // 2, c % 2
        yc = res.results[c]["y"]
        for ch in range(8):
            out[b, 256 * ch + 128 * g:256 * ch + 128 * (g + 1), :] = \
                yc[128 * ch:128 * (ch + 1), :]
    return out

